# revision 8
# baseline (speedup 1.0000x reference)
"""Trainium2 Bass kernel for a 2-layer GATv2 network (N=384, D=512, H=8, F=64).

Strategy (8 NeuronCores, SPMD, full I/O):
  - Destination nodes i are sharded row-wise: core c owns rows [48c, 48c+48).
  - Per layer: every core computes full projections g_l^T (transposed layout,
    [(h,f), j]) plus its own g_r^T columns; the O(N^2 H F) pairwise
    S = relu(g_r[i] + g_l[j]) tiles are built with fused
    tensor_scalar(add,max0) / scalar-engine Relu(bias=col) ops
    ([128=(2h,64f), 384=j] per (i, head-pair)), and reduced over f on the
    TensorEngine with a block-diagonal (0.8*a) weight matrix
    (leaky_relu(z) = 0.2 z + 0.8 relu(z); the destination-side 0.2*a.g_r[i]
    term is constant in j and cancels in softmax, the source-side
    0.2*a.g_l[j] term and the -60*(1-adj) mask ride in as one extra k=12
    matmul into the same PSUM accumulator).
  - e lands in PSUM as [4 i's at partition strides 32 (x8 heads), j]; softmax
    over j (free axis) via reduce_max(negate)/Exp(bias=-max)/reduce_sum/recip.
  - att is transposed per 128-col block on the TensorEngine; aggregation
    att^T @ g_r accumulates straight into a [48, 512] PSUM output tile.
  - Between layers the 48 new rows (transposed) are AllGathered across the
    8 cores (layer 1 needs all of x1).
Host side: shards/preps inputs per core, runs via run_bass_kernel_spmd,
concatenates the 8 [48, 512] output slices.
"""

import numpy as np

import concourse.bass as bass
import concourse.bacc as bacc
import concourse.mybir as mybir
import concourse.tile as tile
from contextlib import ExitStack

N, D, H, F = 384, 512, 8, 64
HF = H * F            # 512
NCORES = 8
R = N // NCORES       # 48 rows per core
NQ = R // 4           # 12 e-tiles per core, 4 destination nodes each
SLOPE = 0.2
MASK_NEG = -60.0
F32 = mybir.dt.float32
ADD = mybir.AluOpType.add
MAX = mybir.AluOpType.max
MIN = mybir.AluOpType.min
AX = mybir.AxisListType.X

# How many of the 16 S-tiles per e-tile are built on the Vector engine
# (the rest go to the Scalar engine as Relu(in + bias)).
S_ON_DVE = 9


def _build_program():
    nc = bacc.Bacc("TRN2", target_bir_lowering=False)

    # ---- I/O ----
    xT = nc.dram_tensor("xT", [D, N], F32, kind="ExternalInput")
    xTm = nc.dram_tensor("xTm", [D, R], F32, kind="ExternalInput")
    xr_m1 = nc.dram_tensor("xr_m1", [R, HF], F32, kind="ExternalInput")
    wl_d = [nc.dram_tensor(f"wl{l}", [D, HF], F32, kind="ExternalInput") for l in (0, 1)]
    wr_d = [nc.dram_tensor(f"wr{l}", [D, HF], F32, kind="ExternalInput") for l in (0, 1)]
    ablk_d = [nc.dram_tensor(f"ablk{l}", [128, 32], F32, kind="ExternalInput") for l in (0, 1)]
    lmsel_d = nc.dram_tensor("lmsel", [12, 128], F32, kind="ExternalInput")
    maskm1_d = nc.dram_tensor("maskm1", [R, N], F32, kind="ExternalInput")
    ident_d = nc.dram_tensor("ident", [128, 128], F32, kind="ExternalInput")
    out_d = nc.dram_tensor("out", [R, HF], F32, kind="ExternalOutput")

    with tile.TileContext(nc) as tc, ExitStack() as ctx:
        P = ctx.enter_context(tc.tile_pool(name="persist", bufs=1))
        WP = ctx.enter_context(tc.tile_pool(name="wpool", bufs=2))
        SP = ctx.enter_context(tc.tile_pool(name="spool", bufs=16))
        AP_ = ctx.enter_context(tc.tile_pool(name="apool", bufs=3))
        SM = ctx.enter_context(tc.tile_pool(name="small", bufs=6))
        TL = ctx.enter_context(tc.tile_pool(name="tails", bufs=2))
        EP = ctx.enter_context(tc.tile_pool(name="epsum", bufs=3, space="PSUM"))
        TP = ctx.enter_context(tc.tile_pool(name="tpsum", bufs=2, space="PSUM"))
        PP = ctx.enter_context(tc.tile_pool(name="ppsum", bufs=2, space="PSUM"))
        YP = ctx.enter_context(tc.tile_pool(name="ypsum", bufs=1, space="PSUM"))
        DR = ctx.enter_context(tc.tile_pool(name="dram", bufs=1, space="DRAM"))

        # ---- persistent SBUF tiles ----
        xt_sb = [P.tile([128, N], F32, name=f"xt{k}", tag=f"xt{k}") for k in range(4)]
        x1t_sb = [P.tile([128, N], F32, name=f"x1t{k}", tag=f"x1t{k}") for k in range(4)]
        xtm_sb = [P.tile([128, R], F32, name=f"xtm{k}", tag=f"xtm{k}") for k in range(4)]
        x1tm_sb = [P.tile([128, R], F32, name=f"x1tm{k}", tag=f"x1tm{k}") for k in range(4)]
        gl_sb = [P.tile([128, N], F32, name=f"gl{k}", tag=f"gl{k}") for k in range(4)]
        grm_sb = [P.tile([128, R], F32, name=f"grm{k}", tag=f"grm{k}") for k in range(4)]
        gr_sb = [P.tile([128, HF], F32, name=f"gr{k}", tag=f"gr{k}") for k in range(3)]
        atT_sb = [P.tile([128, 128 * NQ], F32, name=f"atT{k}", tag=f"atT{k}") for k in range(3)]
        rhs2_sb = [P.tile([12, N], F32, name=f"rhs2{k}", tag=f"rhs2{k}") for k in range(NQ)]
        ablk_sb = [P.tile([128, 32], F32, name=f"ablk{k}", tag=f"ablk{k}") for k in range(2)]
        lmsel_sb = P.tile([12, 128], F32, name="lmsel", tag="lmsel")
        id_sb = P.tile([128, 128], F32, name="ident", tag="ident")
        lT_sb = P.tile([8, N], F32, name="lT", tag="lT")
        xr_sb = P.tile([R, HF], F32, name="xr", tag="xr")
        x1_sb = P.tile([R, HF], F32, name="x1", tag="x1")
        out_sb = P.tile([R, HF], F32, name="outsb", tag="outsb")

        gin_t = DR.tile([D, R], F32, name="gin", tag="gin")
        gout_t = DR.tile([NCORES * D, R], F32, name="gout", tag="gout")

        dma = nc.sync.dma_start

        # ---- static loads ----
        for k in range(4):
            dma(out=xt_sb[k][:], in_=xT[128 * k:128 * (k + 1), :])
            dma(out=xtm_sb[k][:], in_=xTm[128 * k:128 * (k + 1), :])
        for l in range(2):
            dma(out=ablk_sb[l][:], in_=ablk_d[l][:, :])
        dma(out=lmsel_sb[:], in_=lmsel_d[:, :])
        dma(out=id_sb[:], in_=ident_d[:, :])
        dma(out=xr_sb[:], in_=xr_m1[:, :])

        for l in range(2):
            xt_src = xt_sb if l == 0 else x1t_sb
            xtm_src = xtm_sb if l == 0 else x1tm_sb
            abl = ablk_sb[l]

            # weights for this layer
            wl_t = [WP.tile([128, HF], F32, name=f"wl{k}", tag=f"wl{k}") for k in range(4)]
            wr_t = [WP.tile([128, HF], F32, name=f"wr{k}", tag=f"wr{k}") for k in range(4)]
            for k in range(4):
                dma(out=wl_t[k][:], in_=wl_d[l][128 * k:128 * (k + 1), :])
                dma(out=wr_t[k][:], in_=wr_d[l][128 * k:128 * (k + 1), :])

            # --- A1: g_l^T [(hf)-slice hp][128, N] = (x @ W_l)^T ---
            for mt in range(4):
                ps = PP.tile([128, N], F32, name="pp", tag="pp")
                for k in range(4):
                    nc.tensor.matmul(ps[:], wl_t[k][:, 128 * mt:128 * (mt + 1)],
                                     xt_src[k][:], start=(k == 0), stop=(k == 3))
                nc.scalar.copy(gl_sb[mt][:], ps[:])

            # --- A2: my g_r^T columns [(hf)-slice hp][128, R] ---
            for mt in range(4):
                ps = PP.tile([128, R], F32, name="ppm", tag="pp")
                for k in range(4):
                    nc.tensor.matmul(ps[:], wr_t[k][:, 128 * mt:128 * (mt + 1)],
                                     xtm_src[k][:], start=(k == 0), stop=(k == 3))
                nc.scalar.copy(grm_sb[mt][:], ps[:])

            # --- A3: full g_r [j-block jt][128, HF] = x @ W_r ---
            for jt in range(3):
                ps = PP.tile([128, HF], F32, name="pp", tag="pp")
                for k in range(4):
                    nc.tensor.matmul(ps[:], xt_src[k][:, 128 * jt:128 * (jt + 1)],
                                     wr_t[k][:], start=(k == 0), stop=(k == 3))
                nc.scalar.copy(gr_sb[jt][:], ps[:])

            # --- A4: l^T[h, j] = sum_f 0.8 a_f g_l[j,h,f]  (source-side term) ---
            psl = PP.tile([8, N], F32, name="pl", tag="pp")
            for hp in range(4):
                nc.tensor.matmul(psl[:], abl[:, 8 * hp:8 * (hp + 1)], gl_sb[hp][:],
                                 start=(hp == 0), stop=(hp == 3))
            nc.scalar.copy(lT_sb[:], psl[:])
            # rhs2 tiles: rows 0:8 = l^T, rows 8:12 = mask rows for the 4 i's
            for et in range(NQ):
                dma(out=rhs2_sb[et][0:8, :], in_=lT_sb[:])
                dma(out=rhs2_sb[et][8:12, :], in_=maskm1_d[4 * et:4 * et + 4, :])

            # --- B: per 4-destination-node tile: scores + softmax + transpose ---
            for et in range(NQ):
                s_t = [[None] * 4 for _ in range(4)]
                cnt = 0
                for q in range(4):
                    il = 4 * et + q
                    for hp in range(4):
                        st = SP.tile([128, N], F32, name="s", tag="s")
                        col = grm_sb[hp][:, il:il + 1]
                        if cnt % 16 < S_ON_DVE:
                            nc.vector.tensor_scalar(st[:], gl_sb[hp][:], col, 0.0,
                                                    ADD, MAX)
                        else:
                            nc.scalar.activation(st[:], gl_sb[hp][:],
                                                 mybir.ActivationFunctionType.Relu,
                                                 bias=col, scale=1.0)
                        s_t[q][hp] = st
                        cnt += 1

                pe = EP.tile([128, N], F32, name="e", tag="e")
                # l-term + mask first: writes all 128 partitions (start=True)
                nc.tensor.matmul(pe[:], lmsel_sb[:], rhs2_sb[et][:],
                                 start=True, stop=False, skip_group_check=True)
                for q in range(4):
                    for hp in range(4):
                        last = (q == 3) and (hp == 3)
                        nc.tensor.matmul(pe[32 * q:32 * q + 8, :],
                                         abl[:, 8 * hp:8 * (hp + 1)],
                                         s_t[q][hp][:],
                                         start=False, stop=last,
                                         skip_group_check=True,
                                         tile_position=(0, 32 * q))

                # softmax over j (free axis)
                mxn = SM.tile([128, 1], F32, name="mx", tag="mx")
                nc.vector.reduce_max(mxn[:], pe[:], axis=AX, negate=True)
                att = AP_.tile([128, N], F32, name="att", tag="att")
                nc.scalar.activation(att[:], pe[:],
                                     mybir.ActivationFunctionType.Exp,
                                     bias=mxn[:], scale=1.0)
                dsum = SM.tile([128, 1], F32, name="ds", tag="ds")
                nc.vector.reduce_sum(dsum[:], att[:], axis=AX)
                dinv = SM.tile([128, 1], F32, name="di", tag="di")
                nc.vector.reciprocal(dinv[:], dsum[:])
                nc.vector.tensor_scalar(att[:], att[:], dinv[:], None,
                                        mybir.AluOpType.mult)

                for jb in range(3):
                    pt = TP.tile([128, 128], F32, name="t", tag="t")
                    nc.tensor.transpose(pt[:], att[:, 128 * jb:128 * (jb + 1)],
                                        id_sb[:])
                    nc.scalar.copy(atT_sb[jb][:, 128 * et:128 * (et + 1)], pt[:])

            # --- C: aggregation out[i,(h,f)] = sum_j att * g_r ---
            py = YP.tile([R, HF], F32, name="y", tag="y")
            for h in range(H):
                for jb in range(3):
                    lhsT = atT_sb[jb][:].rearrange("p (s t) -> p s t", t=32)[:, :, h]
                    nc.tensor.matmul(py[0:R, 64 * h:64 * (h + 1)], lhsT,
                                     gr_sb[jb][:, 64 * h:64 * (h + 1)],
                                     start=(jb == 0), stop=(jb == 2))

            # --- D: tails ---
            if l == 0:
                # x1 = x + elu(y) = (x - 1) + (y - min(y,0)) + exp(min(y,0))
                mneg = TL.tile([R, HF], F32, name="mneg", tag="mneg")
                nc.vector.tensor_scalar(mneg[:], py[:], 0.0, None, MIN)
                emneg = TL.tile([R, HF], F32, name="emneg", tag="emneg")
                nc.scalar.activation(emneg[:], mneg[:],
                                     mybir.ActivationFunctionType.Exp)
                t1 = TL.tile([R, HF], F32, name="t1", tag="t1")
                nc.vector.tensor_sub(t1[:], py[:], mneg[:])
                nc.vector.tensor_add(t1[:], t1[:], emneg[:])
                nc.vector.tensor_add(x1_sb[:], t1[:], xr_sb[:])
                # transpose x1 rows -> [D, R] pieces, stash + send to gather
                for k in range(4):
                    pt = TP.tile([128, 128], F32, name="t", tag="t")
                    nc.tensor.transpose(pt[0:128, 0:R],
                                        x1_sb[:, 128 * k:128 * (k + 1)],
                                        id_sb[0:R, 0:R])
                    nc.scalar.copy(x1tm_sb[k][:], pt[0:128, 0:R])
                    dma(out=gin_t[128 * k:128 * (k + 1), :], in_=x1tm_sb[k][:])
                nc.gpsimd.collective_compute(
                    "AllGather", mybir.AluOpType.bypass,
                    replica_groups=[list(range(NCORES))],
                    ins=[gin_t[:, :].opt()], outs=[gout_t[:, :].opt()])
                # gathered [8][D, R] -> x1^T tiles [128, N]
                g3 = gout_t[:, :].rearrange("(c d) j -> c d j", c=NCORES)
                for k in range(4):
                    src = g3[:, 128 * k:128 * (k + 1), :].rearrange("c d j -> d c j")
                    dst = x1t_sb[k][:].rearrange("p (c j) -> p c j", c=NCORES)
                    dma(out=dst, in_=src)
            else:
                nc.vector.tensor_add(out_sb[:], x1_sb[:], py[:])
                dma(out=out_d[:, :], in_=out_sb[:])

    nc.compile()
    return nc


_CACHE = {}


def _get_program():
    if "nc" not in _CACHE:
        _CACHE["nc"] = _build_program()
    return _CACHE["nc"]


def _host_prep(x, adj, W_l0, W_r0, a0, W_l1, W_r1, a1):
    x = np.asarray(x, np.float32)
    adjf = (np.asarray(adj).reshape(N, N) != 0).astype(np.float32)
    np.fill_diagonal(adjf, 1.0)
    maskm1 = (MASK_NEG * (1.0 - adjf)).astype(np.float32)

    def ablk(a):
        # k-slice hp of the [HF, H] block-diagonal (0.8*a) matrix, stored
        # side by side: slice hp covers heads (2hp, 2hp+1); within the slice,
        # rows [0,64) belong to head 2hp (column 2hp), rows [64,128) to head
        # 2hp+1 (column 2hp+1).
        a = (1.0 - SLOPE) * np.asarray(a, np.float32)
        m = np.zeros((128, 32), np.float32)
        for hp in range(4):
            for q in range(2):
                m[64 * q:64 * (q + 1), 8 * hp + 2 * hp + q] = a
        return m

    # lmsel: k=12 stationary matrix for the l-term + mask matmul.
    # out[32q+h, j] += 0.25 * lT[h, j] + 1.0 * maskm1[q-th row, j]
    # (lT carries 0.8*l; 0.25*0.8 = 0.2 = SLOPE).
    lmsel = np.zeros((12, 128), np.float32)
    for qq in range(4):
        for h in range(H):
            lmsel[h, 32 * qq + h] = SLOPE / (1.0 - SLOPE)
            lmsel[8 + qq, 32 * qq + h] = 1.0

    xT = np.ascontiguousarray(x.T)
    ident = np.eye(128, dtype=np.float32)
    common = {
        "xT": xT,
        "wl0": np.asarray(W_l0, np.float32), "wr0": np.asarray(W_r0, np.float32),
        "wl1": np.asarray(W_l1, np.float32), "wr1": np.asarray(W_r1, np.float32),
        "ablk0": ablk(a0), "ablk1": ablk(a1),
        "lmsel": lmsel, "ident": ident,
    }
    in_maps = []
    for c in range(NCORES):
        rows = slice(R * c, R * (c + 1))
        m = dict(common)
        m["xTm"] = np.ascontiguousarray(xT[:, rows])
        m["xr_m1"] = np.ascontiguousarray(x[rows] - 1.0)
        m["maskm1"] = np.ascontiguousarray(maskm1[rows])
        in_maps.append(m)
    return in_maps


def kernel(x, adj, W_l0, W_r0, a0, W_l1, W_r1, a1):
    from concourse.bass_utils import run_bass_kernel_spmd

    in_maps = _host_prep(x, adj, W_l0, W_r0, a0, W_l1, W_r1, a1)
    nc = _get_program()
    res = run_bass_kernel_spmd(nc, in_maps, core_ids=list(range(NCORES)))
    out = np.concatenate([res.results[c]["out"] for c in range(NCORES)], axis=0)
    return np.ascontiguousarray(out.astype(np.float32))
